# revision 3
# baseline (speedup 1.0000x reference)
"""RBF Nadaraya-Watson regression kernel for Trainium2, 8-core SPMD.

out = (K @ Ytrain) / (sum_j K + EPS),  K = exp(-||xt - xj||^2 / (2 l^2))

Sharding: Xtest rows split across 8 cores; each core holds full
Xtrain/Ytrain and computes its slice independently (no collectives).

v2: all layout work (transpose, fp8 cast, DoubleRow interleave, row-norm
bias, EPS term) is done host-side in make_in_maps; the device does pure
compute:

  per 128-row train chunk c (256 chunks):
    G^T[j, t] = sum_d Xtrain[j,d] Xtest[t,d]   (PE, fp8 DoubleRow, K=256)
    K0^T[j,t] = exp(2s*G - s*b_j)              (ACT, PSUM->SBUF fp8)
    numer0^T[y, t] += [Y | 1]^T K0^T           (PE, fp8 DoubleRow chunk pairs)
  out[t,:] = numer0[t, 0:64] / (numer0[t, 64] + EPS*exp(s*a_t))

The a_t term is folded multiplicatively (identical in real arithmetic to
exp(-s(a+b)+2sc) / (sum exp(...) + EPS)).
"""

import sys

try:
    import concourse.bass as bass  # noqa: F401
except ImportError:
    sys.path.insert(0, "/opt/trn_rl_repo")

import numpy as np
import ml_dtypes  # noqa: F401

import concourse.bass as bass
import concourse.bacc as bacc
import concourse.tile as tile
from concourse import mybir
from concourse.bass_utils import run_bass_kernel_spmd

AF = mybir.ActivationFunctionType
F32 = mybir.dt.float32
BF16 = mybir.dt.bfloat16
FP8 = mybir.dt.float8e4

EPS = 1e-8


def build(T=1024, NTRAIN=32768, D=256, DY=64, reps=1):
    """Per-core Bass module. T = test rows per core."""
    assert T % 128 == 0 and NTRAIN % 256 == 0 and D == 256
    NCHUNK = NTRAIN // 128   # 256
    TT = T // 128            # 8
    DYP = DY + 1             # Y columns + ones column (denominator)
    YST = 80                 # fp8 DoubleRow: outer free step even + 16B-aligned
    segs = [(s, min(s + 512, T)) for s in range(0, T, 512)]

    nc = bacc.Bacc("TRN2", target_bir_lowering=False, debug=False)
    xtrT_d = nc.dram_tensor("XtrT", [128, NCHUNK, 2, 128], FP8,
                            kind="ExternalInput")
    yp_d = nc.dram_tensor("Yp", [128, NCHUNK, YST], FP8, kind="ExternalInput")
    xteT_d = nc.dram_tensor("XteT", [128, 2, T], FP8, kind="ExternalInput")
    nbias_d = nc.dram_tensor("nbias", [128, NCHUNK], F32, kind="ExternalInput")
    scale2_d = nc.dram_tensor("scale2", [128, 1], F32, kind="ExternalInput")
    epst_d = nc.dram_tensor("epst", [128, TT], F32, kind="ExternalInput")
    identf_d = nc.dram_tensor("identf", [128, 128], F32, kind="ExternalInput")
    out_d = nc.dram_tensor("out", [T, DY], F32, kind="ExternalOutput")

    NPIECE = min(16, NCHUNK)
    CPP = NCHUNK // NPIECE   # chunks per xtrT dma piece
    YPIECE = min(4, NCHUNK)
    CPY = NCHUNK // YPIECE

    with tile.TileContext(nc) as tc:
      for _rep in range(reps):
        with (
            tc.tile_pool(name="persist", bufs=1) as persist,
            tc.tile_pool(name="k0tp", bufs=2) as k0tp,
            tc.tile_pool(name="epi", bufs=2) as epi,
            tc.tile_pool(name="gp_pool", bufs=2, space="PSUM") as gp_pool,
            tc.tile_pool(name="np_pool", bufs=1, space="PSUM") as np_pool,
            tc.tile_pool(name="tp_pool", bufs=2, space="PSUM") as tp_pool,
        ):
            # ---- constant / persistent loads ----
            scale2 = persist.tile([128, 1], F32)
            nc.sync.dma_start(scale2[:], scale2_d.ap())
            nbias = persist.tile([128, NCHUNK], F32)
            nc.sync.dma_start(nbias[:], nbias_d.ap())
            epst = persist.tile([128, TT], F32)
            nc.sync.dma_start(epst[:], epst_d.ap())
            identf = persist.tile([128, 128], F32)
            nc.sync.dma_start(identf[:], identf_d.ap())
            xteT = persist.tile([128, 2, T], FP8)
            nc.sync.dma_start(xteT[:], xteT_d.ap())

            xtrT = persist.tile([128, NCHUNK, 2, 128], FP8)
            yp = persist.tile([128, NCHUNK, YST], FP8)
            # first pieces of Xtrain^T and Y so compute can start early
            nc.sync.dma_start(xtrT[:, 0:CPP], xtrT_d.ap()[:, 0:CPP])
            nc.sync.dma_start(yp[:, 0:CPY], yp_d.ap()[:, 0:CPY])
            for i in range(1, NPIECE):
                nc.sync.dma_start(xtrT[:, i * CPP:(i + 1) * CPP],
                                  xtrT_d.ap()[:, i * CPP:(i + 1) * CPP])
                if i < YPIECE:
                    nc.sync.dma_start(yp[:, i * CPY:(i + 1) * CPY],
                                      yp_d.ap()[:, i * CPY:(i + 1) * CPY])

            # ---- main loop over train chunks ----
            np_ps = np_pool.tile([DYP, T], F32)
            for c in range(NCHUNK):
                gp = gp_pool.tile([128, T], F32, tag="g")
                for (s0, s1) in segs:
                    nc.tensor.matmul(
                        gp[:, s0:s1],
                        lhsT=xtrT[:, c],
                        rhs=xteT[:, :, s0:s1],
                        perf_mode=mybir.MatmulPerfMode.DoubleRow,
                    )
                if c % 2 == 0:
                    k0t2 = k0tp.tile([128, 2, T], FP8, tag="k")
                nc.scalar.activation(
                    k0t2[:, c % 2, :], gp[:], AF.Exp,
                    bias=nbias[:, c:c + 1], scale=scale2[:])
                if c % 2 == 1:
                    for (s0, s1) in segs:
                        nc.tensor.matmul(
                            np_ps[:, s0:s1],
                            lhsT=yp[:, c - 1:c + 1, 0:DYP],
                            rhs=k0t2[:, :, s0:s1],
                            perf_mode=mybir.MatmulPerfMode.DoubleRow,
                            start=(c == 1),
                            stop=(c == NCHUNK - 1),
                            skip_group_check=True,
                        )

            # ---- epilogue: transpose numer^T, divide, store ----
            ncopy = epi.tile([DYP, T], F32, bufs=1)
            nc.vector.tensor_copy(ncopy[:], np_ps[:])
            for tt in range(TT):
                ntp = tp_pool.tile([128, DYP], F32, tag="t")
                nc.tensor.transpose(
                    ntp[:], ncopy[:, tt * 128:(tt + 1) * 128],
                    identf[0:DYP, 0:DYP])
                dvec = epi.tile([128, 1], F32, tag="dv")
                nc.vector.tensor_add(dvec[:], ntp[:, DY:DYP],
                                     epst[:, tt:tt + 1])
                rvec = epi.tile([128, 1], F32, tag="rv")
                nc.vector.reciprocal(rvec[:], dvec[:])
                otile = epi.tile([128, DY], F32, tag="o")
                nc.vector.tensor_scalar_mul(otile[:], ntp[:, 0:DY], rvec[:])
                nc.sync.dma_start(out_d.ap()[tt * 128:(tt + 1) * 128, :],
                                  otile[:])

    nc.compile()
    return nc


_NC_CACHE = {}


def _get_nc(T, NTRAIN, D, DY):
    key = (T, NTRAIN, D, DY)
    if key not in _NC_CACHE:
        _NC_CACHE[key] = build(T=T, NTRAIN=NTRAIN, D=D, DY=DY)
    return _NC_CACHE[key]


def make_in_maps(Ytrain, Xtrain, Xtest, log_lengthscale, n_cores=8):
    f8 = mybir.dt.np(FP8)
    Xtrain = np.asarray(Xtrain, dtype=np.float32)
    Ytrain = np.asarray(Ytrain, dtype=np.float32)
    Xtest = np.asarray(Xtest, dtype=np.float32)
    theta = np.float32(np.asarray(log_lengthscale).reshape(()))
    ntrain, d = Xtrain.shape
    ntest = Xtest.shape[0]
    dy = Ytrain.shape[1]
    nchunk = ntrain // 128
    T = ntest // n_cores
    tt_n = T // 128
    assert d == 256 and dy == 64

    s = np.float32(0.5) * np.exp(np.float32(-2.0) * theta)

    clip = lambda x: np.clip(x, -240.0, 240.0)
    # XtrT[p, c, k, j] = Xtrain[c*128+j, k*128+p]
    XtrT = np.ascontiguousarray(
        clip(Xtrain).reshape(nchunk, 128, 2, 128).transpose(3, 0, 2, 1)
    ).astype(f8)
    # Yp[p, c, 0:64] = Ytrain[c*128+p], Yp[p, c, 64] = 1
    Yp = np.zeros((128, nchunk, 80), dtype=f8)
    Yp[:, :, 0:dy] = clip(Ytrain).reshape(nchunk, 128, dy).transpose(1, 0, 2)
    Yp[:, :, dy] = 1.0
    # nbias[p, c] = -s * ||Xtrain[c*128+p]||^2
    b = (Xtrain * Xtrain).sum(axis=1)
    nbias = np.ascontiguousarray(
        (-s * b).reshape(nchunk, 128).T).astype(np.float32)
    scale2 = np.full((128, 1), 2.0 * s, dtype=np.float32)
    identf = np.eye(128, dtype=np.float32)

    a = (Xtest * Xtest).sum(axis=1)
    with np.errstate(over="ignore"):
        epst_full = (EPS * np.exp((s * a).astype(np.float32))).astype(np.float32)

    maps = []
    for i in range(n_cores):
        Xte = Xtest[i * T:(i + 1) * T]
        # XteT[p, k, t] = Xte[t, k*128+p]
        XteT = np.ascontiguousarray(
            clip(Xte).reshape(T, 2, 128).transpose(2, 1, 0)).astype(f8)
        epst = np.ascontiguousarray(
            epst_full[i * T:(i + 1) * T].reshape(tt_n, 128).T
        ).astype(np.float32)
        maps.append({
            "XtrT": XtrT,
            "Yp": Yp,
            "XteT": XteT,
            "nbias": nbias,
            "scale2": scale2,
            "epst": epst,
            "identf": identf,
        })
    return maps


def kernel(Ytrain, Xtrain, Xtest, log_lengthscale):
    n_cores = 8
    ntest, d = np.asarray(Xtest).shape
    ntrain, dy = np.asarray(Ytrain).shape
    nc = _get_nc(ntest // n_cores, ntrain, d, dy)
    in_maps = make_in_maps(Ytrain, Xtrain, Xtest, log_lengthscale, n_cores)
    res = run_bass_kernel_spmd(nc, in_maps, core_ids=list(range(n_cores)))
    return np.concatenate([res.results[i]["out"] for i in range(n_cores)],
                          axis=0)


# revision 7
# speedup vs baseline: 1.0627x; 1.0627x over previous
"""RBF Nadaraya-Watson regression kernel for Trainium2, 8-core SPMD.

out = (K @ Ytrain) / (sum_j K + EPS),  K = exp(-||xt - xj||^2 / (2 l^2))

Sharding: Xtest rows split across 8 cores; each core holds full
Xtrain/Ytrain and computes its slice independently (no collectives).

v2: all layout work (transpose, fp8 cast, DoubleRow interleave, row-norm
bias, EPS term) is done host-side in make_in_maps; the device does pure
compute:

  per 128-row train chunk c (256 chunks):
    G^T[j, t] = sum_d Xtrain[j,d] Xtest[t,d]   (PE, fp8 DoubleRow, K=256)
    K0^T[j,t] = exp(2s*G - s*b_j)              (ACT, PSUM->SBUF fp8)
    numer0^T[y, t] += [Y | 1]^T K0^T           (PE, fp8 DoubleRow chunk pairs)
  out[t,:] = numer0[t, 0:64] / (numer0[t, 64] + EPS*exp(s*a_t))

The a_t term is folded multiplicatively (identical in real arithmetic to
exp(-s(a+b)+2sc) / (sum exp(...) + EPS)).
"""

import sys

try:
    import concourse.bass as bass  # noqa: F401
except ImportError:
    sys.path.insert(0, "/opt/trn_rl_repo")

import numpy as np
import ml_dtypes  # noqa: F401

import concourse.bass as bass
import concourse.bacc as bacc
import concourse.tile as tile
from concourse import mybir
from concourse.bass_utils import run_bass_kernel_spmd

AF = mybir.ActivationFunctionType
F32 = mybir.dt.float32
BF16 = mybir.dt.bfloat16
FP8 = mybir.dt.float8e4

EPS = 1e-8


def build(T=1024, NTRAIN=32768, D=256, DY=64, reps=1):
    """Per-core Bass module. T = test rows per core."""
    assert T % 128 == 0 and NTRAIN % 256 == 0 and D == 256
    NCHUNK = NTRAIN // 128   # 256
    TT = T // 128            # 8
    DYP = DY + 1             # Y columns + ones column (denominator)
    YST = 80                 # fp8 DoubleRow: outer free step even + 16B-aligned
    segs = [(s, min(s + 512, T)) for s in range(0, T, 512)]

    nc = bacc.Bacc("TRN2", target_bir_lowering=False, debug=False)
    xtrT_d = nc.dram_tensor("XtrT", [128, NCHUNK, 2, 128], FP8,
                            kind="ExternalInput")
    yp_d = nc.dram_tensor("Yp", [128, NCHUNK, YST], FP8, kind="ExternalInput")
    xteT_d = nc.dram_tensor("XteT", [128, 2, T], FP8, kind="ExternalInput")
    nbias_d = nc.dram_tensor("nbias", [128, NCHUNK], F32, kind="ExternalInput")
    scale2_d = nc.dram_tensor("scale2", [128, 1], F32, kind="ExternalInput")
    epst_d = nc.dram_tensor("epst", [128, TT], F32, kind="ExternalInput")
    identf_d = nc.dram_tensor("identf", [128, 128], F32, kind="ExternalInput")
    out_d = nc.dram_tensor("out", [T, DY], F32, kind="ExternalOutput")

    NPIECE = min(16, NCHUNK)
    CPP = NCHUNK // NPIECE   # chunks per xtrT dma piece
    YPIECE = min(4, NCHUNK)
    CPY = NCHUNK // YPIECE

    with tile.TileContext(nc) as tc:
      for _rep in range(reps):
        with (
            tc.tile_pool(name="persist", bufs=1) as persist,
            tc.tile_pool(name="k0tp", bufs=3) as k0tp,
            tc.tile_pool(name="epi", bufs=2) as epi,
            tc.tile_pool(name="gp_pool", bufs=2, space="PSUM") as gp_pool,
            tc.tile_pool(name="np_pool", bufs=1, space="PSUM") as np_pool,
            tc.tile_pool(name="tp_pool", bufs=2, space="PSUM") as tp_pool,
        ):
            # ---- constant / persistent loads ----
            scale2 = persist.tile([128, 1], F32)
            nc.sync.dma_start(scale2[:], scale2_d.ap())
            nbias = persist.tile([128, NCHUNK], F32)
            nc.sync.dma_start(nbias[:], nbias_d.ap())
            epst = persist.tile([128, TT], F32)
            nc.sync.dma_start(epst[:], epst_d.ap())
            identf = persist.tile([128, 128], F32)
            nc.sync.dma_start(identf[:], identf_d.ap())
            xteT = persist.tile([128, 2, T], FP8)
            nc.sync.dma_start(xteT[:], xteT_d.ap())

            xtrT = persist.tile([128, NCHUNK, 2, 128], FP8)
            yp = persist.tile([128, NCHUNK, YST], FP8)
            # first pieces of Xtrain^T and Y so compute can start early
            nc.sync.dma_start(xtrT[:, 0:CPP], xtrT_d.ap()[:, 0:CPP])
            nc.sync.dma_start(yp[:, 0:CPY], yp_d.ap()[:, 0:CPY])
            for i in range(1, NPIECE):
                nc.sync.dma_start(xtrT[:, i * CPP:(i + 1) * CPP],
                                  xtrT_d.ap()[:, i * CPP:(i + 1) * CPP])
                if i < YPIECE:
                    nc.sync.dma_start(yp[:, i * CPY:(i + 1) * CPY],
                                      yp_d.ap()[:, i * CPY:(i + 1) * CPY])

            # ---- main loop over train chunks ----
            # The numer matmul for pair p=(2p,2p+1) is emitted one pair late
            # (during iteration 2p+3) so by the time it reaches the head of
            # the PE FIFO its ACT inputs are long done — otherwise the PE
            # head-of-line blocks on the activation and the engines
            # serialize.
            np_ps = np_pool.tile([DYP, T], F32)
            NPAIR = NCHUNK // 2
            k0bufs = {}

            def numer(p):
                for (s0, s1) in segs:
                    nc.tensor.matmul(
                        np_ps[:, s0:s1],
                        lhsT=yp[:, 2 * p:2 * p + 2, 0:DYP],
                        rhs=k0bufs[p][:, :, s0:s1],
                        perf_mode=mybir.MatmulPerfMode.DoubleRow,
                        start=(p == 0),
                        stop=(p == NPAIR - 1),
                        skip_group_check=True,
                    )

            for c in range(NCHUNK):
                gp = gp_pool.tile([128, T], F32, tag="g")
                for (s0, s1) in segs:
                    nc.tensor.matmul(
                        gp[:, s0:s1],
                        lhsT=xtrT[:, c],
                        rhs=xteT[:, :, s0:s1],
                        perf_mode=mybir.MatmulPerfMode.DoubleRow,
                    )
                if c % 2 == 1 and c >= 3:
                    numer((c - 3) // 2)
                if c % 2 == 0:
                    k0bufs[c // 2] = k0tp.tile([128, 2, T], FP8, tag="k",
                                               name=f"k0t_{c // 2}")
                nc.scalar.activation(
                    k0bufs[c // 2][:, c % 2, :], gp[:], AF.Exp,
                    bias=nbias[:, c:c + 1], scale=scale2[:])
            numer(NPAIR - 1)

            # ---- epilogue: transpose numer^T, divide, store ----
            ncopy = epi.tile([DYP, T], F32, bufs=1)
            nc.vector.tensor_copy(ncopy[:], np_ps[:])
            for tt in range(TT):
                ntp = tp_pool.tile([128, DYP], F32, tag="t")
                nc.tensor.transpose(
                    ntp[:], ncopy[:, tt * 128:(tt + 1) * 128],
                    identf[0:DYP, 0:DYP])
                dvec = epi.tile([128, 1], F32, tag="dv")
                nc.vector.tensor_add(dvec[:], ntp[:, DY:DYP],
                                     epst[:, tt:tt + 1])
                rvec = epi.tile([128, 1], F32, tag="rv")
                nc.vector.reciprocal(rvec[:], dvec[:])
                otile = epi.tile([128, DY], F32, tag="o")
                nc.vector.tensor_scalar_mul(otile[:], ntp[:, 0:DY], rvec[:])
                nc.sync.dma_start(out_d.ap()[tt * 128:(tt + 1) * 128, :],
                                  otile[:])

    nc.compile()
    return nc


_NC_CACHE = {}


def _get_nc(T, NTRAIN, D, DY):
    key = (T, NTRAIN, D, DY)
    if key not in _NC_CACHE:
        _NC_CACHE[key] = build(T=T, NTRAIN=NTRAIN, D=D, DY=DY)
    return _NC_CACHE[key]


def make_in_maps(Ytrain, Xtrain, Xtest, log_lengthscale, n_cores=8):
    f8 = mybir.dt.np(FP8)
    Xtrain = np.asarray(Xtrain, dtype=np.float32)
    Ytrain = np.asarray(Ytrain, dtype=np.float32)
    Xtest = np.asarray(Xtest, dtype=np.float32)
    theta = np.float32(np.asarray(log_lengthscale).reshape(()))
    ntrain, d = Xtrain.shape
    ntest = Xtest.shape[0]
    dy = Ytrain.shape[1]
    nchunk = ntrain // 128
    T = ntest // n_cores
    tt_n = T // 128
    assert d == 256 and dy == 64

    s = np.float32(0.5) * np.exp(np.float32(-2.0) * theta)

    clip = lambda x: np.clip(x, -240.0, 240.0)
    # XtrT[p, c, k, j] = Xtrain[c*128+j, k*128+p]
    XtrT = np.ascontiguousarray(
        clip(Xtrain).reshape(nchunk, 128, 2, 128).transpose(3, 0, 2, 1)
    ).astype(f8)
    # Yp[p, c, 0:64] = Ytrain[c*128+p], Yp[p, c, 64] = 1
    Yp = np.zeros((128, nchunk, 80), dtype=f8)
    Yp[:, :, 0:dy] = clip(Ytrain).reshape(nchunk, 128, dy).transpose(1, 0, 2)
    Yp[:, :, dy] = 1.0
    # nbias[p, c] = -s * ||Xtrain[c*128+p]||^2
    b = (Xtrain * Xtrain).sum(axis=1)
    nbias = np.ascontiguousarray(
        (-s * b).reshape(nchunk, 128).T).astype(np.float32)
    scale2 = np.full((128, 1), 2.0 * s, dtype=np.float32)
    identf = np.eye(128, dtype=np.float32)

    a = (Xtest * Xtest).sum(axis=1)
    with np.errstate(over="ignore"):
        epst_full = (EPS * np.exp((s * a).astype(np.float32))).astype(np.float32)

    maps = []
    for i in range(n_cores):
        Xte = Xtest[i * T:(i + 1) * T]
        # XteT[p, k, t] = Xte[t, k*128+p]
        XteT = np.ascontiguousarray(
            clip(Xte).reshape(T, 2, 128).transpose(2, 1, 0)).astype(f8)
        epst = np.ascontiguousarray(
            epst_full[i * T:(i + 1) * T].reshape(tt_n, 128).T
        ).astype(np.float32)
        maps.append({
            "XtrT": XtrT,
            "Yp": Yp,
            "XteT": XteT,
            "nbias": nbias,
            "scale2": scale2,
            "epst": epst,
            "identf": identf,
        })
    return maps


def kernel(Ytrain, Xtrain, Xtest, log_lengthscale):
    n_cores = 8
    ntest, d = np.asarray(Xtest).shape
    ntrain, dy = np.asarray(Ytrain).shape
    nc = _get_nc(ntest // n_cores, ntrain, d, dy)
    in_maps = make_in_maps(Ytrain, Xtrain, Xtest, log_lengthscale, n_cores)
    res = run_bass_kernel_spmd(nc, in_maps, core_ids=list(range(n_cores)))
    return np.concatenate([res.results[i]["out"] for i in range(n_cores)],
                          axis=0)
